# revision 10
# baseline (speedup 1.0000x reference)
"""Trainium2 Bass kernel for a 6-layer dense transformer discriminator.

Sharding: data-parallel over batch, 2 sequences per core, with
length-specialized "slots": sequences are sorted by their active
token-tile count (ceil(length/128)); slot A holds the 8 longest
(nta tiles each), slot B the 8 shortest (ntb tiles).  Padded tokens
beyond a sequence's length never influence token 0's output (they are
masked as attention keys in every layer), so each core only processes
nta+ntb token tiles instead of 2*4.  The host permutes sequences into
slots and inverse-permutes the output.

Per-core design (token-major fp32 residual, bf16 matmul operands):
  - z (residual) token-major [128,1024] tiles per slot, fp32, SBUF.
  - LayerNorm: bn_stats/bn_aggr; rstd = exp(-0.5*ln(var+eps)) so the
    whole kernel uses a single ACT table set (exp/ln/relu) -> no
    LoadActFuncSet switches.  LN scale folded into following weights.
  - LN output u transposed (PE transpose, bf16) to feature-major uT.
  - QKVO weights DMA'd once per layer in [128,1024] tiles, shared by
    both slots; FFN runs per-slot (frees all 8 PSUM banks for fc2).
  - Attention per head-pair packed with tile_position: scores row-tiled
    (K=64 heads in rows 0-63/64-127), attn@V and the gate-denominator
    col-tiled (M=64 outputs in psum partitions 0-63/64-127) -> pair MMs
    run concurrently on the PE array.
  - Masking folded multiplicatively: V rows gated, denominator = gated
    column sums of exp(scores) via a replicated-gate matmul.
  - Last layer computes only token 0 (narrow NT=8 streams); final head
    is a tiny gen matmul + log_softmax.
"""

import sys
import numpy as np

for _p in ("/opt/trn_rl_repo", "/root/.axon_site/_ro/trn_rl_repo"):
    if _p not in sys.path:
        sys.path.append(_p)

import concourse.bass as bass
import concourse.mybir as mybir
import concourse.tile as tile
import concourse.bacc as bacc
from concourse.masks import make_identity

F32 = mybir.dt.float32
BF16 = mybir.dt.bfloat16
I32 = mybir.dt.int32

# Model dims (hardcoded per problem spec)
B, L, H, V, O, N_LAYERS, N_HEADS = 16, 512, 1024, 32000, 4, 6, 16
DK = H // N_HEADS            # 64
FF = 4 * H                   # 4096
EPS = 1e-5
N_CORES = 8
HC = H // 128                # 8 hidden chunks
FT = FF // 128               # 32 ff tiles
SCALE = 1.0 / np.sqrt(np.float32(DK))
NT = 8                       # padded token-0 width for last-layer compute
AF = mybir.ActivationFunctionType


def build_nc(n_layers, nta, ntb):
    """Per-core Bass kernel with slot tile counts (nta, ntb)."""
    nc = bacc.Bacc()
    slots = [(0, nta), (1, ntb)]

    # ---- DRAM I/O ----
    x_t = nc.dram_tensor("x_ids", [2, L], I32, kind="ExternalInput")
    emb_t = nc.dram_tensor("emb", [V, H], F32, kind="ExternalInput")
    pe_t = nc.dram_tensor("pe", [L, H], F32, kind="ExternalInput")
    gatef_t = nc.dram_tensor("gatef", [2, L], F32, kind="ExternalInput")
    # weights, already transposed + LN-folded on host, bf16
    wqkvo_t = nc.dram_tensor("wqkvo", [n_layers, 4, H, H], BF16, kind="ExternalInput")
    fc1_t = nc.dram_tensor("fc1t", [n_layers, H, FF], BF16, kind="ExternalInput")
    fc2_t = nc.dram_tensor("fc2t", [n_layers, FF, H], BF16, kind="ExternalInput")
    gw_t = nc.dram_tensor("gwt", [H, O], F32, kind="ExternalInput")
    out_t = nc.dram_tensor("out", [2, O], F32, kind="ExternalOutput")
    # scratch for tiny transposes in the final head
    scr1 = nc.dram_tensor("scr1", [2, H], F32)
    scr2 = nc.dram_tensor("scr2", [2, O], F32)

    with tile.TileContext(nc) as tc:
        import contextlib
        ctx = contextlib.ExitStack()
        with ctx:
            const = ctx.enter_context(tc.tile_pool(name="const", bufs=1))
            zres = ctx.enter_context(tc.tile_pool(name="zres", bufs=1))
            act = ctx.enter_context(tc.tile_pool(name="act", bufs=2))
            h1p = ctx.enter_context(tc.tile_pool(name="h1p", bufs=32))
            wpool = ctx.enter_context(tc.tile_pool(name="wpool", bufs=16))
            small = ctx.enter_context(tc.tile_pool(name="small", bufs=4))
            ps = ctx.enter_context(tc.tile_pool(name="ps", bufs=8, space="PSUM"))

            # ---- constants ----
            ident = const.tile([128, 128], BF16)
            make_identity(nc, ident)
            eps_c = const.tile([128, 1], F32)
            nc.vector.memset(eps_c, EPS)
            ones64 = const.tile([128, DK], F32)
            nc.vector.memset(ones64, 1.0)

            # per-slot gate: per-partition scalars [128, 4] and gate
            # replicated over 64 cols (denominator matmul lhsT, bf16)
            gate_sc = {}
            gate_rep = {}
            for s, nt in slots:
                g = const.tile([128, 4], F32, tag=f"gsc{s}", name=f"gsc{s}")
                src = gatef_t[s, :]
                nc.gpsimd.dma_start(out=g, in_=bass.AP(
                    tensor=src.tensor, offset=src.offset, ap=[[1, 128], [128, 4]]))
                gate_sc[s] = g
                for lt in range(nt):
                    gr = const.tile([128, DK], BF16, tag=f"grep{s}_{lt}",
                                    name=f"grep{s}_{lt}")
                    nc.vector.tensor_scalar_mul(out=gr, in0=ones64,
                                                scalar1=g[:, lt:lt + 1])
                    gate_rep[(s, lt)] = gr

            # ---- residual z, embedding gather + positional encoding ----
            z = {}
            for s, nt in slots:
                for lt in range(nt):
                    z[(s, lt)] = zres.tile([128, H], F32, tag=f"z{s}_{lt}",
                                           name=f"z{s}_{lt}")
            idx = {}
            for s, nt in slots:
                ix = small.tile([128, 4], I32, tag=f"idx{s}", name=f"idx{s}")
                src = x_t[s, :]
                nc.gpsimd.dma_start(out=ix, in_=bass.AP(
                    tensor=src.tensor, offset=src.offset, ap=[[1, 128], [128, 4]]))
                idx[s] = ix
            for s, nt in slots:
                for lt in range(nt):
                    et = act.tile([128, H], F32, tag="emb", bufs=2, name="et")
                    nc.gpsimd.indirect_dma_start(
                        out=et, out_offset=None, in_=emb_t[:, :],
                        in_offset=bass.IndirectOffsetOnAxis(
                            ap=idx[s][:, lt:lt + 1], axis=0))
                    pt = act.tile([128, H], F32, tag="emb", bufs=2, name="pt")
                    nc.sync.dma_start(out=pt, in_=pe_t[lt * 128:(lt + 1) * 128, :])
                    nc.vector.tensor_add(out=z[(s, lt)], in0=et, in1=pt)

            def layernorm_T(s, nt, uT):
                """LN (affine folded) of z -> u -> transposed into the 3D
                feature-major tile uT [128, HC, nt*128] (bf16)."""
                mv_all = small.tile([128, nt, 2], F32, tag="bnmv")
                for lt in range(nt):
                    st = small.tile([128, 2, 6], F32, tag="bnst")
                    nc.vector.bn_stats(out=st[:, 0, :], in_=z[(s, lt)][:, 0:512])
                    nc.vector.bn_stats(out=st[:, 1, :], in_=z[(s, lt)][:, 512:1024])
                    nc.vector.bn_aggr(out=mv_all[:, lt, :], in_=st)
                # one batched Sqrt for all tiles (fewer ACT table switches),
                # reciprocal on DVE
                sd = small.tile([128, nt], F32, tag="bnsd")
                nc.scalar.activation(out=sd, in_=mv_all[:, :, 1], func=AF.Sqrt,
                                     bias=eps_c, scale=1.0)
                rs = small.tile([128, nt], F32, tag="bnrs")
                nc.vector.reciprocal_approx_fast(out=rs, in_=sd)
                u_tiles = []
                for lt in range(nt):
                    u = act.tile([128, H], BF16, tag="u", bufs=2)
                    nc.vector.tensor_scalar(
                        out=u, in0=z[(s, lt)], scalar1=mv_all[:, lt, 0:1],
                        scalar2=rs[:, lt:lt + 1],
                        op0=mybir.AluOpType.subtract, op1=mybir.AluOpType.mult)
                    u_tiles.append(u)
                for hk in range(HC):
                    pt_ = ps.tile([128, nt * 128], BF16, tag="ps")
                    for lt in range(nt):
                        nc.tensor.transpose(
                            out=pt_[:, lt * 128:(lt + 1) * 128],
                            in_=u_tiles[lt][:, hk * 128:(hk + 1) * 128],
                            identity=ident)
                    nc.vector.tensor_copy(out=uT[:, hk, :], in_=pt_)

            def new_uT(s, nt, which):
                return act.tile([128, HC, nt * 128], BF16, tag=f"uT{s}",
                                bufs=1, name=f"uT{s}_{which}")

            def load_w_h(w_dram):
                """Load an [H, 1024] weight block as 8 tiles [128, 1024]."""
                wt = []
                for hk in range(HC):
                    w = wpool.tile([128, 1024], BF16, tag="w", bufs=14)
                    nc.sync.dma_start(out=w, in_=w_dram[hk * 128:(hk + 1) * 128, :])
                    wt.append(w)
                return wt

            def proj_fm(wt, uTs, ncq, res):
                """Feature-major projection for both slots, shared weights.
                res: {s: 3D tile [128, HC, ncq[s]]}."""
                for mcg in range(2):
                    for j in range(4):
                        pp = {}
                        for s, nt in slots:
                            pp[s] = ps.tile([128, ncq[s]], F32, tag="ps",
                                            name=f"ppq{s}")
                        for hk in range(HC):
                            wsl = wt[hk][:, mcg * 512 + j * 128:
                                         mcg * 512 + (j + 1) * 128]
                            for s, nt in slots:
                                nc.tensor.matmul(
                                    out=pp[s], lhsT=wsl,
                                    rhs=uTs[s][:, hk, 0:ncq[s]],
                                    start=(hk == 0), stop=(hk == HC - 1))
                        for s, nt in slots:
                            nc.vector.tensor_copy(out=res[s][:, mcg * 4 + j, :],
                                                  in_=pp[s])

            def proj_v(wt, uTs):
                """v token-major [nt][128, H] per slot, gated per token."""
                vt = {s: [act.tile([128, H], BF16, tag="v", name=f"v{s}_{i}",
                                   bufs=7) for i in range(nt)]
                      for s, nt in slots}
                for n in range(2):
                    for s, nt in slots:
                        for lc in range(nt):
                            pp = ps.tile([128, 512], F32, tag="ps")
                            for hk in range(HC):
                                nc.tensor.matmul(
                                    out=pp,
                                    lhsT=uTs[s][:, hk, lc * 128:(lc + 1) * 128],
                                    rhs=wt[hk][:, n * 512:(n + 1) * 512],
                                    start=(hk == 0), stop=(hk == HC - 1))
                            nc.vector.tensor_scalar_mul(
                                out=vt[s][lc][:, n * 512:(n + 1) * 512],
                                in0=pp, scalar1=gate_sc[s][:, lc:lc + 1])
                return vt

            def attention(s, nt, qT, kT, vt, ncq, cT):
                """Packed head-pair attention -> cT [128, HC, ncq]."""
                for t in range(N_HEADS // 2):
                    expS = {}
                    for mt in range(nt):
                        for hh in range(2):
                            po = 64 * hh
                            pss = ps.tile([128, ncq], F32, tag="ps")
                            nc.tensor.matmul(
                                out=pss,
                                lhsT=kT[po:po + 64, t, mt * 128:(mt + 1) * 128],
                                rhs=qT[po:po + 64, t, 0:ncq],
                                start=True, stop=True,
                                tile_position=(po, 0))
                            e = act.tile([128, ncq], BF16, tag="expS", bufs=8)
                            nc.scalar.activation(out=e, in_=pss, func=AF.Exp,
                                                 scale=float(SCALE))
                            expS[(mt, hh)] = e
                    psc = ps.tile([128, ncq], F32, tag="ps")
                    psd = ps.tile([128, ncq], F32, tag="ps")
                    for mt in range(nt):
                        for hh in range(2):
                            po = 64 * hh
                            nc.tensor.matmul(
                                out=psd[po:po + 64, :],
                                lhsT=gate_rep[(s, mt)],
                                rhs=expS[(mt, hh)],
                                start=(mt == 0), stop=(mt == nt - 1),
                                tile_position=(0, po))
                    rr = act.tile([128, ncq], F32, tag="rr", bufs=2)
                    nc.vector.reciprocal_approx_fast(out=rr, in_=psd)
                    for mt in range(nt):
                        for hh in range(2):
                            po = 64 * hh
                            nc.tensor.matmul(
                                out=psc[po:po + 64, :],
                                lhsT=vt[mt][:, (2 * t + hh) * DK:
                                            (2 * t + hh + 1) * DK],
                                rhs=expS[(mt, hh)],
                                start=(mt == 0), stop=(mt == nt - 1),
                                tile_position=(0, po))
                    nc.vector.tensor_tensor(out=cT[:, t, :], in0=psc, in1=rr,
                                            op=mybir.AluOpType.mult)

            def proj_wo_resid(wt, s, nt, cT):
                """z += c @ Wo' for one slot (token-major, fused add)."""
                for n in range(2):
                    for lc in range(nt):
                        pp = ps.tile([128, 512], F32, tag="ps")
                        for hk in range(HC):
                            nc.tensor.matmul(
                                out=pp,
                                lhsT=cT[:, hk, lc * 128:(lc + 1) * 128],
                                rhs=wt[hk][:, n * 512:(n + 1) * 512],
                                start=(hk == 0), stop=(hk == HC - 1))
                        nc.vector.tensor_add(
                            out=z[(s, lc)][:, n * 512:(n + 1) * 512],
                            in0=z[(s, lc)][:, n * 512:(n + 1) * 512],
                            in1=pp)

            def ffn(li, s, nt, u2T):
                """z += fc2(relu(fc1 @ u2)) for one slot."""
                h1 = []
                for mg in range(8):
                    w1 = []
                    for hk in range(HC):
                        w = wpool.tile([128, 512], BF16, tag="wf", bufs=10,
                                       name="w1")
                        nc.sync.dma_start(
                            out=w, in_=fc1_t[li, hk * 128:(hk + 1) * 128,
                                             mg * 512:(mg + 1) * 512])
                        w1.append(w)
                    for j in range(4):
                        pp = ps.tile([128, nt * 128], F32, tag="ps")
                        for hk in range(HC):
                            nc.tensor.matmul(
                                out=pp, lhsT=w1[hk][:, j * 128:(j + 1) * 128],
                                rhs=u2T[:, hk, :],
                                start=(hk == 0), stop=(hk == HC - 1))
                        h = h1p.tile([128, nt * 128], BF16, tag="h1", bufs=32)
                        nc.scalar.activation(out=h, in_=pp, func=AF.Relu)
                        h1.append(h)
                # fc2: all nt*2 accumulators live (<= 8 banks per slot)
                po = {}
                for lc in range(nt):
                    for n in range(2):
                        po[(lc, n)] = ps.tile([128, 512], F32, tag="ps",
                                              name=f"po{lc}_{n}")
                for k in range(FT):
                    w2 = wpool.tile([128, 1024], BF16, tag="w2", bufs=4,
                                    name="w2")
                    nc.sync.dma_start(
                        out=w2, in_=fc2_t[li, k * 128:(k + 1) * 128, :])
                    for lc in range(nt):
                        for n in range(2):
                            nc.tensor.matmul(
                                out=po[(lc, n)],
                                lhsT=h1[k][:, lc * 128:(lc + 1) * 128],
                                rhs=w2[:, n * 512:(n + 1) * 512],
                                start=(k == 0), stop=(k == FT - 1))
                for lc in range(nt):
                    for n in range(2):
                        nc.vector.tensor_add(
                            out=z[(s, lc)][:, n * 512:(n + 1) * 512],
                            in0=z[(s, lc)][:, n * 512:(n + 1) * 512],
                            in1=po[(lc, n)])

            def wo_tok0(wt, s, cT8):
                """z[rows 0:NT] += (c @ Wo')[0:NT] for one slot."""
                for n in range(2):
                    pp = ps.tile([NT, 512], F32, tag="ps")
                    for hk in range(HC):
                        nc.tensor.matmul(
                            out=pp, lhsT=cT8[:, hk, 0:NT],
                            rhs=wt[hk][:, n * 512:(n + 1) * 512],
                            start=(hk == 0), stop=(hk == HC - 1))
                    nc.vector.tensor_add(
                        out=z[(s, 0)][0:NT, n * 512:(n + 1) * 512],
                        in0=z[(s, 0)][0:NT, n * 512:(n + 1) * 512], in1=pp)

            def ln2_tok0(s):
                """LN of z rows 0:NT -> transposed u2T0 [128, HC*NT] bf16."""
                st = small.tile([128, 2, 6], F32, tag="bnst")
                nc.vector.bn_stats(out=st[0:NT, 0, :], in_=z[(s, 0)][0:NT, 0:512])
                nc.vector.bn_stats(out=st[0:NT, 1, :], in_=z[(s, 0)][0:NT, 512:1024])
                mv = small.tile([128, 2], F32, tag="bnmv2")
                nc.vector.bn_aggr(out=mv[0:NT, :], in_=st[0:NT, :, :])
                sd = small.tile([128, 1], F32, tag="bnsd2")
                nc.scalar.activation(out=sd[0:NT, :], in_=mv[0:NT, 1:2],
                                     func=AF.Sqrt, bias=eps_c[0:NT, :], scale=1.0)
                rs = small.tile([128, 1], F32, tag="bnrs2")
                nc.vector.reciprocal(out=rs[0:NT, :], in_=sd[0:NT, :])
                u2 = act.tile([128, H], BF16, tag="u", bufs=2)
                nc.vector.tensor_scalar(
                    out=u2[0:NT, :], in0=z[(s, 0)][0:NT, :],
                    scalar1=mv[0:NT, 0:1], scalar2=rs[0:NT, :],
                    op0=mybir.AluOpType.subtract, op1=mybir.AluOpType.mult)
                pt_ = ps.tile([128, HC * NT], BF16, tag="ps")
                for hk in range(HC):
                    nc.tensor.transpose(
                        out=pt_[:, hk * NT:(hk + 1) * NT],
                        in_=u2[0:NT, hk * 128:(hk + 1) * 128],
                        identity=ident[0:NT, 0:NT])
                u2T0 = small.tile([128, HC * NT], BF16, tag=f"u2t0_{s}",
                                  name=f"u2t0_{s}")
                nc.vector.tensor_copy(out=u2T0, in_=pt_)
                return u2T0

            def ffn_tok0(li, u2T0s):
                """z[rows 0:NT] += ffn on the narrow token-0 slice, both
                slots sharing weight loads."""
                h1n = {s: [] for s, _ in slots}
                for mg in range(8):
                    w1 = []
                    for hk in range(HC):
                        w = wpool.tile([128, 512], BF16, tag="wf", bufs=10,
                                       name="w1")
                        nc.sync.dma_start(
                            out=w, in_=fc1_t[li, hk * 128:(hk + 1) * 128,
                                             mg * 512:(mg + 1) * 512])
                        w1.append(w)
                    for j in range(4):
                        pp = {}
                        for s, nt in slots:
                            pp[s] = ps.tile([128, NT], F32, tag="ps",
                                            name=f"ppn{s}")
                        for hk in range(HC):
                            wsl = w1[hk][:, j * 128:(j + 1) * 128]
                            for s, nt in slots:
                                nc.tensor.matmul(
                                    out=pp[s], lhsT=wsl,
                                    rhs=u2T0s[s][:, hk * NT:(hk + 1) * NT],
                                    start=(hk == 0), stop=(hk == HC - 1))
                        for s, nt in slots:
                            h = small.tile([128, NT], BF16, tag="h1n", bufs=70)
                            nc.scalar.activation(out=h, in_=pp[s], func=AF.Relu)
                            h1n[s].append(h)
                po2 = {}
                for s, nt in slots:
                    for n in range(2):
                        po2[(s, n)] = ps.tile([NT, 512], F32, tag="ps",
                                              name=f"po2_{s}_{n}")
                for k in range(FT):
                    w2 = wpool.tile([128, 1024], BF16, tag="w2", bufs=4,
                                    name="w2")
                    nc.sync.dma_start(
                        out=w2, in_=fc2_t[li, k * 128:(k + 1) * 128, :])
                    for s, nt in slots:
                        for n in range(2):
                            nc.tensor.matmul(
                                out=po2[(s, n)], lhsT=h1n[s][k][:, 0:NT],
                                rhs=w2[:, n * 512:(n + 1) * 512],
                                start=(k == 0), stop=(k == FT - 1))
                for s, nt in slots:
                    for n in range(2):
                        nc.vector.tensor_add(
                            out=z[(s, 0)][0:NT, n * 512:(n + 1) * 512],
                            in0=z[(s, 0)][0:NT, n * 512:(n + 1) * 512],
                            in1=po2[(s, n)])

            # ---- main layer loop ----
            for li in range(n_layers):
                last = (li == n_layers - 1)
                uTs = {}
                for s, nt in slots:
                    uTs[s] = new_uT(s, nt, f"ln1_{li}")
                    layernorm_T(s, nt, uTs[s])
                ncq = {s: (NT if last else nt * 128) for s, nt in slots}
                qTs = {s: act.tile([128, HC, ncq[s]], BF16, tag=f"qT{s}",
                                   bufs=1, name=f"qT{s}_{li}")
                       for s, nt in slots}
                kTs = {s: act.tile([128, HC, nt * 128], BF16, tag=f"kT{s}",
                                   bufs=1, name=f"kT{s}_{li}")
                       for s, nt in slots}
                wq = load_w_h(wqkvo_t[li, 0])
                proj_fm(wq, uTs, ncq, qTs)
                wk = load_w_h(wqkvo_t[li, 1])
                proj_fm(wk, uTs, {s: nt * 128 for s, nt in slots}, kTs)
                wv = load_w_h(wqkvo_t[li, 2])
                vts = proj_v(wv, uTs)
                wo = load_w_h(wqkvo_t[li, 3])
                cTs = {s: act.tile([128, HC, ncq[s]], BF16, tag=f"cT{s}",
                                   bufs=1, name=f"cT{s}_{li}")
                       for s, nt in slots}
                if last:
                    for s, nt in slots:
                        attention(s, nt, qTs[s], kTs[s], vts[s], ncq[s], cTs[s])
                        wo_tok0(wo, s, cTs[s])
                    u2T0s = {}
                    for s, nt in slots:
                        u2T0s[s] = ln2_tok0(s)
                    ffn_tok0(li, u2T0s)
                else:
                    for s, nt in slots:
                        attention(s, nt, qTs[s], kTs[s], vts[s], ncq[s], cTs[s])
                        proj_wo_resid(wo, s, nt, cTs[s])
                    for s, nt in slots:
                        u2T = new_uT(s, nt, f"ln2_{li}")
                        layernorm_T(s, nt, u2T)
                        ffn(li, s, nt, u2T)

            # ---- final head (token 0 only per slot) ----
            gw_sb = const.tile([128, HC, O], F32)
            nc.sync.dma_start(out=gw_sb,
                              in_=gw_t.rearrange("(kt p) o -> p kt o", p=128))
            for s, nt in slots:
                st = small.tile([128, 2, 6], F32, tag="bnst")
                nc.vector.bn_stats(out=st[0:1, 0, :], in_=z[(s, 0)][0:1, 0:512])
                nc.vector.bn_stats(out=st[0:1, 1, :], in_=z[(s, 0)][0:1, 512:1024])
                mv = small.tile([128, 2], F32, tag="bnmv2")
                nc.vector.bn_aggr(out=mv[0:1, :], in_=st[0:1, :, :])
                sd = small.tile([128, 1], F32, tag="bnsd2")
                nc.scalar.activation(out=sd[0:1, :], in_=mv[0:1, 1:2],
                                     func=AF.Sqrt, bias=eps_c[0:1, :], scale=1.0)
                rs = small.tile([128, 1], F32, tag="bnrs2")
                nc.vector.reciprocal(out=rs[0:1, :], in_=sd[0:1, :])
                u0 = act.tile([128, H], F32, tag="emb", bufs=2, name="u0")
                nc.vector.tensor_scalar(
                    out=u0[0:1, :], in0=z[(s, 0)][0:1, :],
                    scalar1=mv[0:1, 0:1], scalar2=rs[0:1, :],
                    op0=mybir.AluOpType.subtract, op1=mybir.AluOpType.mult)
                nc.sync.dma_start(out=scr1[s, :], in_=u0[0:1, :])
                z0T = small.tile([128, HC], F32, tag="z0t")
                nc.sync.dma_start(
                    out=z0T, in_=scr1[s, :].rearrange("(k p) -> p k", p=128))
                pg = ps.tile([O, 1], F32, tag="ps")
                for k in range(HC):
                    nc.tensor.matmul(out=pg, lhsT=gw_sb[:, k, :],
                                     rhs=z0T[:, k:k + 1],
                                     start=(k == 0), stop=(k == HC - 1))
                lgc = small.tile([O, 1], F32, tag="lgc")
                nc.vector.tensor_copy(out=lgc, in_=pg)
                nc.sync.dma_start(out=scr2[s, :], in_=lgc[:, 0])
                lgr = small.tile([1, O], F32, tag="lgr")
                nc.sync.dma_start(out=lgr[0:1, :], in_=scr2[s, :])
                ex = small.tile([1, O], F32, tag="ex")
                ssum = small.tile([1, 1], F32, tag="ssum")
                nc.scalar.activation(out=ex[0:1, :], in_=lgr[0:1, :],
                                     func=AF.Exp, accum_out=ssum[0:1, :])
                lse = small.tile([1, 1], F32, tag="lse")
                nc.scalar.activation(out=lse[0:1, :], in_=ssum[0:1, :],
                                     func=AF.Ln)
                orow = small.tile([1, O], F32, tag="orow")
                nc.vector.tensor_scalar(
                    out=orow[0:1, :], in0=lgr[0:1, :], scalar1=lse[0:1, :],
                    scalar2=None, op0=mybir.AluOpType.subtract)
                nc.sync.dma_start(out=out_t[s, :], in_=orow[0:1, :])

    nc.compile()
    return nc


def _pos_enc():
    pos = np.arange(L, dtype=np.float32)[:, None]
    dim = np.arange(H // 2, dtype=np.float32)[None, :]
    div = np.float32(10000.0) ** (dim / np.float32(H))
    pe = np.zeros((L, H), np.float32)
    pe[:, 0::2] = np.sin(pos / div)
    pe[:, 1::2] = np.cos(pos / div)
    return pe


def prep_host(x, length, emb, Wq, Wk, Wv, Wo, ln1_w, ln1_b, ln2_w, ln2_b,
              fc1_w, fc1_b, fc2_w, fc2_b, gen_ln_w, gen_ln_b, gen_w, gen_b,
              n_layers=N_LAYERS):
    """Fold LN affine into weights (bf16); build slot assignment and the
    per-core input maps.  Returns (in_maps, perm, nta, ntb) where perm[r]
    is the original sequence index of concatenated output row r."""
    import ml_dtypes
    bf16 = ml_dtypes.bfloat16
    x = np.asarray(x).astype(np.int32)
    length = np.asarray(length).astype(np.int64)
    f32 = lambda a: np.ascontiguousarray(np.asarray(a, dtype=np.float32))
    emb = f32(emb)
    Wq, Wk, Wv, Wo = f32(Wq), f32(Wk), f32(Wv), f32(Wo)
    ln1_w, ln1_b, ln2_w, ln2_b = f32(ln1_w), f32(ln1_b), f32(ln2_w), f32(ln2_b)
    fc1_w, fc1_b = f32(fc1_w), f32(fc1_b)
    fc2_w, fc2_b = f32(fc2_w), f32(fc2_b)
    gen_ln_w, gen_ln_b, gen_w, gen_b = (f32(gen_ln_w), f32(gen_ln_b),
                                        f32(gen_w), f32(gen_b))

    # biases must be zero (they are, for the reference setup_inputs) --
    # the kernel folds LN scale into weights and drops additive biases.
    for i in range(n_layers):
        assert not np.any(ln1_b[i] @ Wq[i].T), "nonzero q bias unsupported"
        assert not np.any(ln1_b[i] @ Wk[i].T), "nonzero k bias unsupported"
        assert not np.any(ln1_b[i] @ Wv[i].T), "nonzero v bias unsupported"
        assert not np.any(fc1_b[i] + fc1_w[i] @ ln2_b[i]), "nonzero fc1 bias unsupported"
        assert not np.any(fc2_b[i]), "nonzero fc2 bias unsupported"
    assert not np.any(gen_b + gen_w @ gen_ln_b), "nonzero gen bias unsupported"

    wqkvo = np.empty((n_layers, 4, H, H), bf16)
    fc1t = np.empty((n_layers, H, FF), bf16)
    fc2t = np.empty((n_layers, FF, H), bf16)
    for i in range(n_layers):
        wqkvo[i, 0] = (ln1_w[i][:, None] * Wq[i].T).astype(bf16)
        wqkvo[i, 1] = (ln1_w[i][:, None] * Wk[i].T).astype(bf16)
        wqkvo[i, 2] = (ln1_w[i][:, None] * Wv[i].T).astype(bf16)
        wqkvo[i, 3] = Wo[i].T.astype(bf16)
        fc1t[i] = (ln2_w[i][:, None] * fc1_w[i].T).astype(bf16)
        fc2t[i] = fc2_w[i].T.astype(bf16)
    gwt = np.ascontiguousarray((gen_w * gen_ln_w[None, :]).T)  # [H, O]

    pe = _pos_enc()
    gate_full = (np.arange(L)[None, :] < length[:, None]).astype(np.float32)

    # slot assignment: sort by active tile count desc (stable), slot A =
    # 8 longest, slot B = 8 shortest
    ntiles = np.ceil(length / 128).astype(int)
    order = np.argsort(-ntiles, kind="stable")
    slotA, slotB = order[:N_CORES], order[N_CORES:]
    nta, ntb = int(ntiles[slotA[0]]), int(ntiles[slotB[0]])

    in_maps = []
    perm = []
    for c in range(N_CORES):
        sa, sb = int(slotA[c]), int(slotB[c])
        perm += [sa, sb]
        in_maps.append({
            "x_ids": np.ascontiguousarray(x[[sa, sb]]),
            "emb": emb,
            "pe": pe,
            "gatef": np.ascontiguousarray(gate_full[[sa, sb]]),
            "wqkvo": wqkvo,
            "fc1t": fc1t,
            "fc2t": fc2t,
            "gwt": gwt,
        })
    return in_maps, perm, nta, ntb


_NC_CACHE = {}


def _get_nc(n_layers=N_LAYERS, nta=4, ntb=3):
    key = (n_layers, nta, ntb)
    if key not in _NC_CACHE:
        _NC_CACHE[key] = build_nc(n_layers, nta, ntb)
    return _NC_CACHE[key]


def kernel(**inputs) -> np.ndarray:
    from concourse.bass_utils import run_bass_kernel_spmd
    in_maps, perm, nta, ntb = prep_host(**inputs)
    nc = _get_nc(N_LAYERS, nta, ntb)
    res = run_bass_kernel_spmd(nc, in_maps, core_ids=list(range(N_CORES)),
                               trace=False)
    raw = np.concatenate([res.results[c]["out"] for c in range(N_CORES)], axis=0)
    out = np.empty((B, O), np.float32)
    out[perm] = raw
    return out
